# revision 13
# baseline (speedup 1.0000x reference)
"""Multi-head attention (B=2, S=2048, E=1024, H=16, D=64) on 8 trn2 cores — v3.

Sharding: head-parallel. Core c owns heads {2c, 2c+1} for both batches
(contiguous 128-wide column slice of x / of the attention output).
Each core computes q/k/v + attention for its 2 heads and a
contraction-sharded partial of the output projection (its 128 rows of
W_out^T); the host sums the 8 partials and adds the bias.

v3 vs v2 (both changes microbenched on HW):
- fp16 operands everywhere (PSUM accumulation stays fp32). HW rates:
  K=128 [128,512] matmul 291-293ns (f32r: 311-374), exp [128,1024]
  1024ns fp16-out (1265 f32r-out). fp16 keeps 10 mantissa bits so the
  extra rounding (~5e-4 per tensor) stays far under the 2e-2 gate.
- PE row-tile pairing for the K=64 logits matmuls: a lone K=64 matmul
  streams at only ~1.0 ns/row (503-515ns per [*,512] regardless of
  dtype), but ALTERNATING matmuls between row-tiles (0,0) and (64,0)
  overlaps their streams: 186.7ns each. Heads 0/1 of this core live on
  partitions 0-63 / 64-127 of qT/kT, so interleaving the two heads'
  logits matmuls gets the pairing for free. Hence the unit structure:
  one unit = (batch, q-quarter, k-tile) covering BOTH heads:
  lg[128,1024] = [h0 512 cols | h1 512 cols] -> ONE [128,1024] exp ->
  pt -> 2 AV accumulate matmuls (one per head).

Schedule skeleton (from v2): one continuous 128-unit train
(logits+exp), with AV emission lagging via a FIFO so the in-order PE
queue never hard-stalls, and ALL other work (projections, v staging,
output projection, DMA) as micro side-steps, at most one per unit.

PSUM budget (8 banks): lg tag 6 bufs x [128,512] (one per head per
unit, 3 units deep; side-step matmuls share this rotation so their
PSUM->SBUF copy latency never blocks the train), acc tag 2 bufs x
[128,512]. The shaped RESID drains each phase's AV tail fast (lag
9->4 by kt=15) so norm(p) is emitted ~6 units before phase p+1's
first AV needs p's accumulator banks; the AV hole at phase starts
(kt 0-4) hosts two side-steps per unit instead of one.

The AV stationary is [v_h | ones*64] (128 cols): PSUM accumulator rows
64-127 hold the softmax denominator replicated 64x for free; normalize
is reciprocal+multiply on DVE (v2's HW-verified 32-partition staging
for the cross-quadrant sums move).
"""

import numpy as np

B, S, E, H, D = 2, 2048, 1024, 16, 64
NCORES = 8
SCALE = 0.125   # 1/sqrt(64)
NT = S // 128   # 16 k tiles
QH = 4          # q quarters per batch
QW = S // QH    # 512
NPH = B * QH    # 8 phases; phase p = b*4 + qh
NUNITS = NPH * NT  # 128 units; unit u = p*NT + kt

# AV-emission lag (units) per kt: drain the AV fifo down to RESID[kt]
# entries at unit kt of each phase. Falling tail = fast drain at phase
# end (norm lands early); the refilling fifo leaves an AV hole at
# kt 0-4 that absorbs two side-steps per unit.
RESID = [9, 9, 9, 9, 9, 9, 9, 9, 9, 9, 9, 9, 9, 8, 6, 4]

_PROG = None


def _build_program(loop_n=0, variant=""):
    import concourse.mybir as mybir
    import concourse.tile as tile
    from concourse import bacc
    from concourse._compat import get_trn_type

    fp32 = mybir.dt.float32
    fp16 = mybir.dt.float16
    AF = mybir.ActivationFunctionType
    Alu = mybir.AluOpType

    nc = bacc.Bacc(get_trn_type() or "TRN2", target_bir_lowering=False)
    xst = nc.dram_tensor("xst", [B, 128, S], fp16, kind="ExternalInput")
    bdq = nc.dram_tensor("bdq", [128, 128], fp16, kind="ExternalInput")
    bdk = nc.dram_tensor("bdk", [128, 128], fp16, kind="ExternalInput")
    bdv = nc.dram_tensor("bdv", [128, 128], fp16, kind="ExternalInput")
    wot = nc.dram_tensor("wot", [128, E], fp16, kind="ExternalInput")
    part = nc.dram_tensor("part", [B, S, E], fp16, kind="ExternalOutput")
    dbg = {}
    if "debug" in variant:
        dbg["qT"] = nc.dram_tensor("dbg_qT", [128, S], fp16, kind="ExternalOutput")
        dbg["kT"] = nc.dram_tensor("dbg_kT", [128, S], fp16, kind="ExternalOutput")
        dbg["va"] = nc.dram_tensor("dbg_va", [128, NT * 256], fp16, kind="ExternalOutput")
        dbg["at"] = nc.dram_tensor("dbg_at", [128, S], fp16, kind="ExternalOutput")
        dbg["pt"] = nc.dram_tensor("dbg_pt", [128, 1024], fp16, kind="ExternalOutput")
        dbg["inv"] = nc.dram_tensor("dbg_inv", [64, QW], fp32, kind="ExternalOutput")

    with tile.TileContext(nc) as tc:
        with (
            tc.tile_pool(name="consts", bufs=1) as consts,
            tc.tile_pool(name="xhT", bufs=2) as xhT_pool,
            tc.tile_pool(name="qT", bufs=2) as qT_pool,
            tc.tile_pool(name="kT", bufs=2) as kT_pool,
            tc.tile_pool(name="vaug", bufs=2) as vaug_pool,
            tc.tile_pool(name="pt", bufs=12) as pt_pool,
            tc.tile_pool(name="attnb", bufs=2) as attnb_pool,
            tc.tile_pool(name="inv", bufs=2) as inv_pool,
            tc.tile_pool(name="outsb", bufs=4) as out_pool,
            tc.tile_pool(name="ps", bufs=3, space="PSUM") as ps_pool,
        ):
            # tiny exp at t=0 so the ACT table set loads while DMAs run
            warm = consts.tile([1, 64], fp32)
            nc.gpsimd.memset(warm[:], 0.0)
            nc.scalar.activation(warm[:, 0:8], warm[:, 0:8], AF.Exp)

            # DMA queue is serial: q/k weights + batch-0 x first (they gate
            # the first exp), v weight next, wot last (first needed ~20us in)
            bdq_sb = consts.tile([128, 128], fp16)
            nc.sync.dma_start(bdq_sb[:], bdq[:])
            bdk_sb = consts.tile([128, 128], fp16)
            nc.sync.dma_start(bdk_sb[:], bdk[:])
            bdv_sb = consts.tile([128, 128], fp16)
            wot_sb = consts.tile([128, E], fp16)

            def pe_warmup():
                # PE_HAM flips to full clock after ~3.4us of activity; burn
                # tiny matmuls while the input DMAs run so the projection
                # matmuls on the critical path run at full clock
                dm = ps_pool.tile([64, 64], fp32, tag="lg", name="warmps")
                for _ in range(14):
                    nc.tensor.matmul(dm[:], warm[0:1, :], warm[0:1, :])

            xhT_t, qT_t, kT_t, vaug_t, attnb_t = {}, {}, {}, {}, {}
            wmap = {"q": (bdq_sb, qT_t), "k": (bdk_sb, kT_t)}

            def copy_(eng, dst, src):
                if eng is nc.scalar:
                    nc.scalar.copy(dst, src)
                elif eng is nc.gpsimd:
                    nc.gpsimd.tensor_copy(dst, src)
                else:
                    nc.vector.tensor_copy(dst, src)

            def stage_b(b):
                """Allocate xhT/vaug for batch b; start its DMAs + memset."""
                xhT = xhT_pool.tile([128, S], fp16, tag="xhT", name=f"xhT{b}")
                xhT_t[b] = xhT
                if b == 0:  # split: unblocks the first q/k projection sooner
                    nc.sync.dma_start(xhT[:, 0:512], xst[b, :, 0:512])
                    nc.sync.dma_start(xhT[:, 512:S], xst[b, :, 512:S])
                else:
                    nc.sync.dma_start(xhT[:, 0:S], xst[b, :, 0:S])
                vaug = vaug_pool.tile([128, NT * 2, 128], fp16, tag="vaug",
                                      name=f"va{b}")
                vaug_t[b] = vaug
                nc.gpsimd.memset(vaug[:], 1.0)
                qT_t[b] = qT_pool.tile([128, S], fp16, tag="qT", name=f"qT{b}")
                kT_t[b] = kT_pool.tile([128, S], fp16, tag="kT", name=f"kT{b}")
                attnb_t[b] = attnb_pool.tile([128, S], fp16, tag="attnb",
                                             name=f"at{b}")
                if "noav" in variant:  # timing ablation: attnb needs a writer
                    nc.gpsimd.memset(attnb_t[b][:], 0.0)

            def qk_chunk(b, which, c, eng, n=1):
                """Project q or k for n 512-position quarters of batch b
                (n<=2; one PSUM tile + one copy)."""
                w_sb, dst_t = wmap[which]
                lo = c * QW
                ps = ps_pool.tile([128, QW * n], fp32, tag="lg",
                                  name=f"{which}p{b}_{c}")
                for i in range(n):
                    nc.tensor.matmul(ps[:, i * QW:(i + 1) * QW], w_sb[:],
                                     xhT_t[b][:, lo + i * QW:lo + (i + 1) * QW])
                copy_(eng, dst_t[b][:, lo:lo + n * QW], ps[:])

            def v_tile(b, st):
                """v for one 128-wide k-tile of batch b -> vaug v-columns
                of both heads via ONE strided copy (dst [128,2,64])."""
                ps = ps_pool.tile([128, 2, 64], fp32, tag="lg", name=f"vp{b}_{st}")
                nc.tensor.matmul(ps[:], xhT_t[b][:, st * 128:(st + 1) * 128],
                                 bdv_sb[:])
                nc.vector.tensor_copy(
                    vaug_t[b][:, st * 2:st * 2 + 2, 0:64], ps[:])

            def outproj(b, st, ec, eng):
                """One 512-col mm of the output-projection partial; the
                copy-out is ONE [128,1024] DVE op at ec==1."""
                if "nofinal" in variant:
                    return
                if ec == 0:
                    outproj.osb = out_pool.tile([128, E], fp16, tag="o",
                                                name=f"o{b}_{st}")
                    outproj.ps = ps_pool.tile([128, E], fp32, tag="lg",
                                              name=f"fp{b}_{st}")
                nc.tensor.matmul(
                    outproj.ps[:, ec * 512:(ec + 1) * 512],
                    attnb_t[b][:, st * 128:(st + 1) * 128],
                    wot_sb[:, ec * 512:(ec + 1) * 512],
                )
                if ec == 1:
                    if "opscalar" in variant:
                        eng = nc.scalar
                    copy_(eng, outproj.osb[:], outproj.ps[:])
                    if "nodma" not in variant:
                        nc.sync.dma_start(part[b, st * 128:(st + 1) * 128, :],
                                          outproj.osb[:])

            acc_t = {}

            def emit_av(p, kt, pt):
                if "noav" in variant:  # timing ablation: tiny AV
                    b = p >> 2
                    for h in range(2):
                        nc.tensor.matmul(
                            acc_t[p][h][:, 0:8],
                            vaug_t[b][:, kt * 2 + h],
                            pt[:, h * 512:h * 512 + 8],
                            start=(kt == 0),
                            stop=(kt == NT - 1),
                        )
                    return
                b = p >> 2
                vaug = vaug_t[b]
                for h in range(2):
                    nc.tensor.matmul(
                        acc_t[p][h][:],
                        vaug[:, kt * 2 + h],
                        pt[:, h * 512:(h + 1) * 512],
                        start=(kt == 0),
                        stop=(kt == NT - 1),
                    )

            def emit_norm(p):
                if "noav" in variant or "nonorm" in variant:
                    return  # noav: attnb pre-written by stage_b memset
                # NOTE: 64-partition DVE ops with input/output in different
                # PSUM/SBUF quadrants produce garbage on HW (sim doesn't
                # model the bank->quadrant routing). Only <=32-partition
                # cross-quadrant moves are HW-verified, so stage the sums
                # down to partitions 0-63 with two 32-partition copies,
                # then run reciprocal+multiply fully aligned (v2 pattern).
                b, qh = p >> 2, p & 3
                for h in range(2):
                    aw = acc_t[p][h]
                    sums = inv_pool.tile([64, QW], fp32, tag="sums",
                                         name=f"sm{p}_{h}")
                    inv = inv_pool.tile([64, QW], fp32, tag="inv",
                                       name=f"inv{p}_{h}")
                    if dbg and p == 0 and h == 0:
                        emit_norm.dump = inv
                    if "normscalar" in variant:
                        nc.scalar.copy(sums[0:32, :], aw[64:96, :])
                        nc.scalar.copy(sums[32:64, :], aw[96:128, :])
                    else:
                        nc.vector.tensor_copy(sums[0:32, :], aw[64:96, :])
                        nc.vector.tensor_copy(sums[32:64, :], aw[96:128, :])
                    if "norecip" in variant:
                        nc.vector.tensor_copy(inv[:], sums[:])
                    else:
                        # ~51-ULP fast reciprocal (inputs are sums of
                        # positive exps -- no denorm/inf edge cases)
                        nc.vector.reciprocal_approx_fast(inv[:], sums[:])
                    nc.vector.tensor_tensor(
                        attnb_t[b][h * 64:(h + 1) * 64,
                                   qh * QW:(qh + 1) * QW],
                        aw[0:64, :],
                        inv[:],
                        Alu.mult,
                    )

            # ---- schedule ----
            def emit_schedule(warmup=True):
                stage_b(0)
                if warmup:
                    pe_warmup()
                nc.sync.dma_start(bdv_sb[:], bdv[:])
                nc.sync.dma_start(wot_sb[:], wot[:])
                qk_chunk(0, "k", 0, nc.scalar)  # ACT idle pre-train
                qk_chunk(0, "q", 0, nc.scalar)
                for st in range(4):
                    v_tile(0, st)

                side = []  # (min_unit, closure)

                def add(u0, fn, *a):
                    side.append((u0, lambda: fn(*a)))

                # b0 k quarters: chunk c needed by unit 4c (kt tiles 4c..4c+3)
                add(0, qk_chunk, 0, "k", 1, nc.vector, 2)  # k1+k2: need u4/u8
                for n, st in enumerate((4, 5, 6)):
                    add(1 + n, v_tile, 0, st)
                add(4, qk_chunk, 0, "k", 3, nc.vector)     # need u12
                for n, st in enumerate((7, 8, 9)):
                    add(5 + n, v_tile, 0, st)
                add(9, qk_chunk, 0, "q", 1, nc.vector)     # need u16
                for n, st in enumerate((10, 11, 12)):
                    add(10 + n, v_tile, 0, st)
                for n, st in enumerate((13, 14, 15)):
                    add(13 + n, v_tile, 0, st)
                add(16, stage_b, 1)                        # b1 x DMA
                # outproj(0, st in 4p..4p+3) after norm(p) (fifo drains phase
                # p's last AV ~RESID units into phase p+1)
                n = 0
                for st in range(4):
                    for ec in range(2):
                        add(25 + n, outproj, 0, st, ec, nc.vector)
                        n += 1
                add(26, qk_chunk, 0, "q", 2, nc.vector, 2)  # q2+q3: u32/u48
                # b1 v tiles (xhT(1) landed by ~u24)
                for n, st in enumerate(range(8)):
                    add(34 + n, v_tile, 1, st)
                n = 0
                for st in range(4, 8):
                    for ec in range(2):
                        add(42 + n, outproj, 0, st, ec, nc.vector)
                        n += 1
                add(50, qk_chunk, 1, "k", 0, nc.vector, 2)  # need u64/u68
                add(54, qk_chunk, 1, "k", 2, nc.vector, 2)  # need u72/u76
                for n, st in enumerate(range(8, 16)):
                    add(50 + n, v_tile, 1, st)
                add(58, qk_chunk, 1, "q", 0, nc.vector)     # need u64
                n = 0
                for st in range(8, 12):
                    for ec in range(2):
                        add(66 + n, outproj, 0, st, ec, nc.vector)
                        n += 1
                n = 0
                for st in range(12, 16):
                    for ec in range(2):
                        add(74 + n, outproj, 0, st, ec, nc.vector)
                        n += 1
                add(75, qk_chunk, 1, "q", 1, nc.vector)     # need u80
                add(88, qk_chunk, 1, "q", 2, nc.vector, 2)  # q2+q3: u96/u112
                n = 0
                for st in range(4):
                    for ec in range(2):
                        add(89 + n, outproj, 1, st, ec, nc.vector)
                        n += 1
                n = 0
                for st in range(4, 8):
                    for ec in range(2):
                        add(106 + n, outproj, 1, st, ec, nc.vector)
                        n += 1
                n = 0
                for st in range(8, 12):
                    for ec in range(2):
                        add(121 + n, outproj, 1, st, ec, nc.vector)
                        n += 1
                side.sort(key=lambda it: it[0])
                side.reverse()  # pop() from the end

                fifo = []
                for u in range(NUNITS):
                    p, kt = divmod(u, NT)
                    b, qh = p >> 2, p & 3
                    if kt == 0:
                        acc_t[p] = tuple(
                            ps_pool.tile([128, 512], fp32, tag="acc",
                                         bufs=2, name=f"acc{p}_{h}")
                            for h in range(2))
                    pt = pt_pool.tile([128, QW * 2], fp16, tag="pt",
                                      name=f"pt{p}_{kt}")
                    # one [128,1024] lg tile for BOTH heads: the h1 matmul
                    # carries no separate PSUM-buf wait, so the row-tile
                    # paired pair always issues back-to-back, and ONE exp
                    # covers both heads (fewer cross-engine handoffs)
                    lg = ps_pool.tile([128, QW * 2], fp32, tag="lg",
                                      name=f"lg{p}_{kt}")
                    for h in range(2):
                        if "nologits" in variant:  # timing ablation: tiny mm
                            nc.tensor.matmul(
                                lg[:, h * 512:h * 512 + 8],
                                kT_t[b][h * 64:(h + 1) * 64,
                                        kt * 128:(kt + 1) * 128],
                                qT_t[b][h * 64:(h + 1) * 64,
                                        qh * QW:qh * QW + 8],
                            )
                        else:
                            nc.tensor.matmul(
                                lg[:, h * 512:(h + 1) * 512],
                                kT_t[b][h * 64:(h + 1) * 64,
                                        kt * 128:(kt + 1) * 128],
                                qT_t[b][h * 64:(h + 1) * 64,
                                        qh * QW:(qh + 1) * QW],
                            )
                    if "noexp" in variant:  # timing ablation: tiny exp
                        nc.scalar.activation(pt[:, 0:8], lg[:, 0:8],
                                             AF.Exp, scale=SCALE)
                    else:
                        nc.scalar.activation(pt[:], lg[:], AF.Exp, scale=SCALE)
                    if dbg and p == 0 and kt == 0:
                        nc.sync.dma_start(dbg["pt"][:], pt[:])
                    fifo.append((p, kt, pt))
                    while len(fifo) > RESID[kt]:
                        fp, fkt, fpt = fifo.pop(0)
                        emit_av(fp, fkt, fpt)
                        if fkt == NT - 1:
                            emit_norm(fp)
                    nside = 2 if kt <= 4 else (0 if kt >= 14 else 1)
                    for _ in range(nside):
                        if side and side[-1][0] <= u:
                            side.pop()[1]()

                while fifo:
                    fp, fkt, fpt = fifo.pop(0)
                    emit_av(fp, fkt, fpt)
                    if fkt == NT - 1:
                        emit_norm(fp)
                while side:  # shouldn't happen, but never drop work
                    side.pop()[1]()
                if dbg:
                    nc.sync.dma_start(dbg["inv"][:], emit_norm.dump[:])
                    nc.sync.dma_start(dbg["qT"][:], qT_t[0][:])
                    nc.sync.dma_start(dbg["kT"][:], kT_t[0][:])
                    nc.sync.dma_start(dbg["va"][:], vaug_t[0][:])
                    nc.sync.dma_start(dbg["at"][:], attnb_t[0][:])
                dm = ps_pool.tile([64, 64], fp32, tag="lg", name="tailwarm")
                for _ in range(10):
                    nc.tensor.matmul(dm[:], warm[0:1, :], warm[0:1, :])
                for n, st in enumerate(range(12, 16)):  # b1 tail
                    outproj(1, st, 0, nc.scalar if n % 2 == 0 else nc.vector)
                    outproj(1, st, 1, nc.vector if n % 2 == 0 else nc.scalar)

            if loop_n > 1:
                pe_warmup()  # once: PE stays warm across iterations
                with tc.For_i(0, loop_n, 1, hint_engines=tuple(nc.engines)):
                    emit_schedule(warmup=False)
            else:
                emit_schedule()

    nc.compile()
    return nc


def _get_program():
    global _PROG
    if _PROG is None:
        import os
        _PROG = _build_program(int(os.environ.get("BASS_MHA_LOOP", "0")),
                               os.environ.get("BASS_MHA_VARIANT", ""))
    return _PROG


def make_in_maps(x, W_qkv, W_out):
    x = np.ascontiguousarray(np.asarray(x, dtype=np.float32))
    W_qkv = np.asarray(W_qkv, dtype=np.float32)
    W_out = np.asarray(W_out, dtype=np.float32)

    def bd(w):  # block_diag(w.T, w.T)
        out = np.zeros((128, 128), dtype=np.float16)
        out[0:64, 0:64] = w.T
        out[64:128, 64:128] = w.T
        return out

    bdq = bd(W_qkv[0:64])
    bdk = bd(W_qkv[64:128])
    bdv = bd(W_qkv[128:192])
    WoT = np.ascontiguousarray(W_out.T.astype(np.float16))
    in_maps = []
    for c in range(NCORES):
        in_maps.append({
            "xst": np.ascontiguousarray(
                x[:, :, c * 128:(c + 1) * 128].transpose(0, 2, 1)
            ).astype(np.float16),
            "bdq": bdq,
            "bdk": bdk,
            "bdv": bdv,
            "wot": np.ascontiguousarray(WoT[c * 128:(c + 1) * 128, :]),
        })
    return in_maps


def kernel(x, W_qkv, W_out, b_out, _trace=False):
    from concourse import bass_utils

    nc = _get_program()
    in_maps = make_in_maps(x, W_qkv, W_out)
    res = bass_utils.run_bass_kernel_spmd(
        nc, in_maps, core_ids=list(range(NCORES)), trace=_trace
    )
    acc = np.zeros((B, S, E), dtype=np.float64)
    for r in res.results:
        acc += r["part"]
    acc += np.asarray(b_out, dtype=np.float64)
    out = acc.astype(np.float32)
    if _trace:
        return out, res
    return out


# revision 16
# speedup vs baseline: 1.4091x; 1.4091x over previous
"""Multi-head attention (B=2, S=2048, E=1024, H=16, D=64) on 8 trn2 cores — v3.

Sharding: head-parallel. Core c owns heads {2c, 2c+1} for both batches
(contiguous 128-wide column slice of x / of the attention output).
Each core computes q/k/v + attention for its 2 heads and a
contraction-sharded partial of the output projection (its 128 rows of
W_out^T); the host sums the 8 partials and adds the bias.

v3 vs v2 (both changes microbenched on HW):
- fp16 operands everywhere (PSUM accumulation stays fp32). HW rates:
  K=128 [128,512] matmul 291-293ns (f32r: 311-374), exp [128,1024]
  1024ns fp16-out (1265 f32r-out). fp16 keeps 10 mantissa bits so the
  extra rounding (~5e-4 per tensor) stays far under the 2e-2 gate.
- PE row-tile pairing for the K=64 logits matmuls: a lone K=64 matmul
  streams at only ~1.0 ns/row (503-515ns per [*,512] regardless of
  dtype), but ALTERNATING matmuls between row-tiles (0,0) and (64,0)
  overlaps their streams: 186.7ns each. Heads 0/1 of this core live on
  partitions 0-63 / 64-127 of qT/kT, so interleaving the two heads'
  logits matmuls gets the pairing for free. Hence the unit structure:
  one unit = (batch, q-quarter, k-tile) covering BOTH heads:
  lg[128,1024] = [h0 512 cols | h1 512 cols] -> ONE [128,1024] exp ->
  pt -> 2 AV accumulate matmuls (one per head).

Schedule skeleton (from v2): one continuous 128-unit train
(logits+exp), with AV emission lagging via a FIFO so the in-order PE
queue never hard-stalls, and ALL other work (projections, v staging,
output projection, DMA) as micro side-steps, at most one per unit.

PSUM budget (8 banks): lg tag 6 bufs x [128,512] (one per head per
unit, 3 units deep; side-step matmuls share this rotation so their
PSUM->SBUF copy latency never blocks the train), acc tag 2 bufs x
[128,512]. The shaped RESID drains each phase's AV tail fast (lag
9->4 by kt=15) so norm(p) is emitted ~6 units before phase p+1's
first AV needs p's accumulator banks; the AV hole at phase starts
(kt 0-4) hosts two side-steps per unit instead of one.

The AV stationary is [v_h | ones*64] (128 cols): PSUM accumulator rows
64-127 hold the softmax denominator replicated 64x for free; normalize
is reciprocal+multiply on DVE (v2's HW-verified 32-partition staging
for the cross-quadrant sums move).
"""

import numpy as np

B, S, E, H, D = 2, 2048, 1024, 16, 64
NCORES = 8
SCALE = 0.125   # 1/sqrt(64)
NT = S // 128   # 16 k tiles
QH = 4          # q quarters per batch
QW = S // QH    # 512
NPH = B * QH    # 8 phases; phase p = b*4 + qh
NUNITS = NPH * NT  # 128 units; unit u = p*NT + kt

# AV-emission lag (units) per kt: drain the AV fifo down to RESID[kt]
# entries at unit kt of each phase. Falling tail = fast drain at phase
# end (norm lands early); the refilling fifo leaves an AV hole at
# kt 0-4 that absorbs two side-steps per unit.
RESID = [9, 9, 9, 9, 9, 9, 9, 9, 9, 9, 9, 9, 9, 8, 6, 4]

_PROG = None


def _build_program(loop_n=0, variant=""):
    import concourse.mybir as mybir
    import concourse.tile as tile
    from concourse import bacc
    from concourse._compat import get_trn_type

    fp32 = mybir.dt.float32
    fp16 = mybir.dt.float16
    AF = mybir.ActivationFunctionType
    Alu = mybir.AluOpType

    nc = bacc.Bacc(get_trn_type() or "TRN2", target_bir_lowering=False)
    xst = nc.dram_tensor("xst", [B, 128, S], fp16, kind="ExternalInput")
    bdq = nc.dram_tensor("bdq", [128, 128], fp16, kind="ExternalInput")
    bdk = nc.dram_tensor("bdk", [128, 128], fp16, kind="ExternalInput")
    bdv = nc.dram_tensor("bdv", [128, 128], fp16, kind="ExternalInput")
    wot = nc.dram_tensor("wot", [128, E], fp16, kind="ExternalInput")
    part = nc.dram_tensor("part", [B, S, E], fp16, kind="ExternalOutput")
    dbg = {}
    if "debug" in variant:
        dbg["qT"] = nc.dram_tensor("dbg_qT", [128, S], fp16, kind="ExternalOutput")
        dbg["kT"] = nc.dram_tensor("dbg_kT", [128, S], fp16, kind="ExternalOutput")
        dbg["va"] = nc.dram_tensor("dbg_va", [128, NT * 256], fp16, kind="ExternalOutput")
        dbg["at"] = nc.dram_tensor("dbg_at", [128, S], fp16, kind="ExternalOutput")
        dbg["pt"] = nc.dram_tensor("dbg_pt", [128, 1024], fp16, kind="ExternalOutput")
        dbg["inv"] = nc.dram_tensor("dbg_inv", [64, QW], fp32, kind="ExternalOutput")

    with tile.TileContext(nc) as tc:
        with (
            tc.tile_pool(name="consts", bufs=1) as consts,
            tc.tile_pool(name="xhT", bufs=2) as xhT_pool,
            tc.tile_pool(name="qT", bufs=2) as qT_pool,
            tc.tile_pool(name="kT", bufs=2) as kT_pool,
            tc.tile_pool(name="vaug", bufs=2) as vaug_pool,
            tc.tile_pool(name="pt", bufs=12) as pt_pool,
            tc.tile_pool(name="attnb", bufs=2) as attnb_pool,
            tc.tile_pool(name="inv", bufs=2) as inv_pool,
            tc.tile_pool(name="outsb", bufs=4) as out_pool,
            tc.tile_pool(name="ps", bufs=3, space="PSUM") as ps_pool,
        ):
            # tiny exp at t=0 so the ACT table set loads while DMAs run
            warm = consts.tile([1, 64], fp32)
            nc.gpsimd.memset(warm[:], 0.0)
            nc.scalar.activation(warm[:, 0:8], warm[:, 0:8], AF.Exp)

            # DMA queue is serial: q/k weights + batch-0 x first (they gate
            # the first exp), v weight next, wot last (first needed ~20us in)
            bdq_sb = consts.tile([128, 128], fp16)
            nc.sync.dma_start(bdq_sb[:], bdq[:])
            bdk_sb = consts.tile([128, 128], fp16)
            nc.sync.dma_start(bdk_sb[:], bdk[:])
            bdv_sb = consts.tile([128, 128], fp16)
            wot_sb = consts.tile([128, E], fp16)

            def pe_warmup():
                # PE_HAM flips to full clock after ~3.4us of activity; burn
                # tiny matmuls while the input DMAs run so the projection
                # matmuls on the critical path run at full clock
                dm = ps_pool.tile([64, 64], fp32, tag="lg", name="warmps")
                for _ in range(14):
                    nc.tensor.matmul(dm[:], warm[0:1, :], warm[0:1, :])

            xhT_t, qT_t, kT_t, vaug_t, attnb_t = {}, {}, {}, {}, {}
            wmap = {"q": (bdq_sb, qT_t), "k": (bdk_sb, kT_t)}

            def copy_(eng, dst, src):
                if eng is nc.scalar:
                    nc.scalar.copy(dst, src)
                elif eng is nc.gpsimd:
                    nc.gpsimd.tensor_copy(dst, src)
                else:
                    nc.vector.tensor_copy(dst, src)

            def stage_b(b):
                """Allocate xhT/vaug for batch b; start its DMAs + memset."""
                xhT = xhT_pool.tile([128, S], fp16, tag="xhT", name=f"xhT{b}")
                xhT_t[b] = xhT
                if b == 0:  # split: unblocks the first q/k projection sooner
                    nc.sync.dma_start(xhT[:, 0:512], xst[b, :, 0:512])
                    nc.sync.dma_start(xhT[:, 512:S], xst[b, :, 512:S])
                else:
                    nc.sync.dma_start(xhT[:, 0:S], xst[b, :, 0:S])
                vaug = vaug_pool.tile([128, NT * 2, 128], fp16, tag="vaug",
                                      name=f"va{b}")
                vaug_t[b] = vaug
                nc.gpsimd.memset(vaug[:], 1.0)
                qT_t[b] = qT_pool.tile([128, S], fp16, tag="qT", name=f"qT{b}")
                kT_t[b] = kT_pool.tile([128, S], fp16, tag="kT", name=f"kT{b}")
                attnb_t[b] = attnb_pool.tile([128, S], fp16, tag="attnb",
                                             name=f"at{b}")
                if "noav" in variant:  # timing ablation: attnb needs a writer
                    nc.gpsimd.memset(attnb_t[b][:], 0.0)

            def qk_chunk(b, which, c, eng, n=1):
                """Project q or k for n 512-position quarters of batch b
                (n<=2; one PSUM tile + one copy)."""
                w_sb, dst_t = wmap[which]
                lo = c * QW
                ps = ps_pool.tile([128, QW * n], fp32, tag="lg",
                                  name=f"{which}p{b}_{c}")
                for i in range(n):
                    nc.tensor.matmul(ps[:, i * QW:(i + 1) * QW], w_sb[:],
                                     xhT_t[b][:, lo + i * QW:lo + (i + 1) * QW])
                copy_(eng, dst_t[b][:, lo:lo + n * QW], ps[:])

            def v_tile(b, st):
                """v for one 128-wide k-tile of batch b -> vaug v-columns
                of both heads via ONE strided copy (dst [128,2,64])."""
                ps = ps_pool.tile([128, 2, 64], fp32, tag="lg", name=f"vp{b}_{st}")
                nc.tensor.matmul(ps[:], xhT_t[b][:, st * 128:(st + 1) * 128],
                                 bdv_sb[:])
                nc.vector.tensor_copy(
                    vaug_t[b][:, st * 2:st * 2 + 2, 0:64], ps[:])

            def outproj(b, st, ec, eng):
                """One 512-col mm of the output-projection partial; the
                copy-out is ONE [128,1024] DVE op at ec==1."""
                if "nofinal" in variant:
                    return
                if ec == 0:
                    outproj.osb = out_pool.tile([128, E], fp16, tag="o",
                                                name=f"o{b}_{st}")
                    outproj.ps = ps_pool.tile([128, E], fp32, tag="lg",
                                              name=f"fp{b}_{st}")
                nc.tensor.matmul(
                    outproj.ps[:, ec * 512:(ec + 1) * 512],
                    attnb_t[b][:, st * 128:(st + 1) * 128],
                    wot_sb[:, ec * 512:(ec + 1) * 512],
                )
                if ec == 1:
                    if "opscalar" in variant:
                        eng = nc.scalar
                    copy_(eng, outproj.osb[:], outproj.ps[:])
                    if "nodma" not in variant:
                        nc.sync.dma_start(part[b, st * 128:(st + 1) * 128, :],
                                          outproj.osb[:])

            acc_t = {}

            def emit_av(p, kt, pt):
                if "noav" in variant:  # timing ablation: tiny AV
                    b = p >> 2
                    for h in range(2):
                        nc.tensor.matmul(
                            acc_t[p][h][:, 0:8],
                            vaug_t[b][:, kt * 2 + h],
                            pt[:, h * 512:h * 512 + 8],
                            start=(kt == 0),
                            stop=(kt == NT - 1),
                        )
                    return
                b = p >> 2
                vaug = vaug_t[b]
                for h in range(2):
                    nc.tensor.matmul(
                        acc_t[p][h][:],
                        vaug[:, kt * 2 + h],
                        pt[:, h * 512:(h + 1) * 512],
                        start=(kt == 0),
                        stop=(kt == NT - 1),
                    )

            def emit_norm(p):
                if "noav" in variant or "nonorm" in variant:
                    return  # noav: attnb pre-written by stage_b memset
                # NOTE: 64-partition DVE ops with input/output in different
                # PSUM/SBUF quadrants produce garbage on HW (sim doesn't
                # model the bank->quadrant routing). Only <=32-partition
                # cross-quadrant moves are HW-verified, so stage the sums
                # down to partitions 0-63 with two 32-partition copies,
                # then run reciprocal+multiply fully aligned (v2 pattern).
                b, qh = p >> 2, p & 3
                for h in range(2):
                    aw = acc_t[p][h]
                    sums = inv_pool.tile([64, QW], fp32, tag="sums",
                                         name=f"sm{p}_{h}")
                    inv = inv_pool.tile([64, QW], fp32, tag="inv",
                                       name=f"inv{p}_{h}")
                    if dbg and p == 0 and h == 0:
                        emit_norm.dump = inv
                    if "normscalar" in variant:
                        nc.scalar.copy(sums[0:32, :], aw[64:96, :])
                        nc.scalar.copy(sums[32:64, :], aw[96:128, :])
                    else:
                        nc.vector.tensor_copy(sums[0:32, :], aw[64:96, :])
                        nc.vector.tensor_copy(sums[32:64, :], aw[96:128, :])
                    if "norecip" in variant:
                        nc.vector.tensor_copy(inv[:], sums[:])
                    else:
                        # ~51-ULP fast reciprocal (inputs are sums of
                        # positive exps -- no denorm/inf edge cases)
                        nc.vector.reciprocal_approx_fast(inv[:], sums[:])
                    nc.vector.tensor_tensor(
                        attnb_t[b][h * 64:(h + 1) * 64,
                                   qh * QW:(qh + 1) * QW],
                        aw[0:64, :],
                        inv[:],
                        Alu.mult,
                    )

            # ---- schedule ----
            def emit_schedule(warmup=True):
                stage_b(0)
                if warmup:
                    pe_warmup()
                nc.sync.dma_start(bdv_sb[:], bdv[:])
                nc.sync.dma_start(wot_sb[:], wot[:])
                qk_chunk(0, "k", 0, nc.scalar)  # ACT idle pre-train
                qk_chunk(0, "q", 0, nc.scalar)
                for st in range(4):
                    v_tile(0, st)

                side = []  # (min_unit, closure)

                def add(u0, fn, *a):
                    side.append((u0, lambda: fn(*a)))

                # b0 k quarters: chunk c needed by unit 4c (kt tiles 4c..4c+3)
                add(0, qk_chunk, 0, "k", 1, nc.vector, 2)  # k1+k2: need u4/u8
                for n, st in enumerate((4, 5, 6)):
                    add(1 + n, v_tile, 0, st)
                add(4, qk_chunk, 0, "k", 3, nc.vector)     # need u12
                for n, st in enumerate((7, 8, 9)):
                    add(5 + n, v_tile, 0, st)
                add(9, qk_chunk, 0, "q", 1, nc.vector)     # need u16
                for n, st in enumerate((10, 11, 12)):
                    add(10 + n, v_tile, 0, st)
                for n, st in enumerate((13, 14, 15)):
                    add(13 + n, v_tile, 0, st)
                add(16, stage_b, 1)                        # b1 x DMA
                # outproj(0, st in 4p..4p+3) after norm(p) (fifo drains phase
                # p's last AV ~RESID units into phase p+1)
                n = 0
                for st in range(4):
                    for ec in range(2):
                        add(25 + n, outproj, 0, st, ec, nc.vector)
                        n += 1
                add(26, qk_chunk, 0, "q", 2, nc.vector, 2)  # q2+q3: u32/u48
                # b1 v tiles (xhT(1) landed by ~u24)
                for n, st in enumerate(range(8)):
                    add(34 + n, v_tile, 1, st)
                n = 0
                for st in range(4, 8):
                    for ec in range(2):
                        add(42 + n, outproj, 0, st, ec, nc.vector)
                        n += 1
                add(50, qk_chunk, 1, "k", 0, nc.vector, 2)  # need u64/u68
                add(54, qk_chunk, 1, "k", 2, nc.vector, 2)  # need u72/u76
                for n, st in enumerate(range(8, 16)):
                    add(50 + n, v_tile, 1, st)
                add(58, qk_chunk, 1, "q", 0, nc.vector)     # need u64
                n = 0
                for st in range(8, 12):
                    for ec in range(2):
                        add(66 + n, outproj, 0, st, ec, nc.vector)
                        n += 1
                n = 0
                for st in range(12, 16):
                    for ec in range(2):
                        add(74 + n, outproj, 0, st, ec, nc.vector)
                        n += 1
                add(75, qk_chunk, 1, "q", 1, nc.vector)     # need u80
                add(88, qk_chunk, 1, "q", 2, nc.vector, 2)  # q2+q3: u96/u112
                n = 0
                for st in range(4):
                    for ec in range(2):
                        add(89 + n, outproj, 1, st, ec, nc.vector)
                        n += 1
                n = 0
                for st in range(4, 8):
                    for ec in range(2):
                        add(106 + n, outproj, 1, st, ec, nc.vector)
                        n += 1
                n = 0
                for st in range(8, 12):
                    for ec in range(2):
                        add(121 + n, outproj, 1, st, ec, nc.vector)
                        n += 1
                side.sort(key=lambda it: it[0])
                side.reverse()  # pop() from the end

                fifo = []
                for u in range(NUNITS):
                    p, kt = divmod(u, NT)
                    b, qh = p >> 2, p & 3
                    if kt == 0:
                        acc_t[p] = tuple(
                            ps_pool.tile([128, 512], fp32, tag="acc",
                                         bufs=2, name=f"acc{p}_{h}")
                            for h in range(2))
                    pt = pt_pool.tile([128, QW * 2], fp16, tag="pt",
                                      name=f"pt{p}_{kt}")
                    # one [128,1024] lg tile for BOTH heads: the h1 matmul
                    # carries no separate PSUM-buf wait, so the row-tile
                    # paired pair always issues back-to-back, and ONE exp
                    # covers both heads (fewer cross-engine handoffs)
                    lg = ps_pool.tile([128, QW * 2], fp32, tag="lg",
                                      name=f"lg{p}_{kt}")
                    for h in range(2):
                        if "nologits" in variant:  # timing ablation: tiny mm
                            nc.tensor.matmul(
                                lg[:, h * 512:h * 512 + 8],
                                kT_t[b][h * 64:(h + 1) * 64,
                                        kt * 128:(kt + 1) * 128],
                                qT_t[b][h * 64:(h + 1) * 64,
                                        qh * QW:qh * QW + 8],
                            )
                        else:
                            nc.tensor.matmul(
                                lg[:, h * 512:(h + 1) * 512],
                                kT_t[b][h * 64:(h + 1) * 64,
                                        kt * 128:(kt + 1) * 128],
                                qT_t[b][h * 64:(h + 1) * 64,
                                        qh * QW:(qh + 1) * QW],
                            )
                    if "noexp" in variant:  # timing ablation: tiny exp
                        nc.scalar.activation(pt[:, 0:8], lg[:, 0:8],
                                             AF.Exp, scale=SCALE)
                    else:
                        nc.scalar.activation(pt[:], lg[:], AF.Exp, scale=SCALE)
                    if dbg and p == 0 and kt == 0:
                        nc.sync.dma_start(dbg["pt"][:], pt[:])
                    fifo.append((p, kt, pt))
                    while len(fifo) > RESID[kt]:
                        fp, fkt, fpt = fifo.pop(0)
                        emit_av(fp, fkt, fpt)
                        if fkt == NT - 1:
                            emit_norm(fp)
                    nside = 2 if kt <= 4 else (0 if kt >= 14 else 1)
                    for _ in range(nside):
                        if side and side[-1][0] <= u:
                            side.pop()[1]()

                while fifo:
                    fp, fkt, fpt = fifo.pop(0)
                    emit_av(fp, fkt, fpt)
                    if fkt == NT - 1:
                        emit_norm(fp)
                while side:  # shouldn't happen, but never drop work
                    side.pop()[1]()
                if dbg:
                    nc.sync.dma_start(dbg["inv"][:], emit_norm.dump[:])
                    nc.sync.dma_start(dbg["qT"][:], qT_t[0][:])
                    nc.sync.dma_start(dbg["kT"][:], kT_t[0][:])
                    nc.sync.dma_start(dbg["va"][:], vaug_t[0][:])
                    nc.sync.dma_start(dbg["at"][:], attnb_t[0][:])
                dm = ps_pool.tile([64, 64], fp32, tag="lg", name="tailwarm")
                for _ in range(10):
                    nc.tensor.matmul(dm[:], warm[0:1, :], warm[0:1, :])
                for n, st in enumerate(range(12, 16)):  # b1 tail
                    outproj(1, st, 0, nc.scalar if n % 2 == 0 else nc.vector)
                    outproj(1, st, 1, nc.vector if n % 2 == 0 else nc.scalar)

            if loop_n > 1:
                pe_warmup()  # once: PE stays warm across iterations
                with tc.For_i(0, loop_n, 1, hint_engines=tuple(nc.engines)):
                    emit_schedule(warmup=False)
            else:
                emit_schedule()

    nc.compile()
    return nc


def _get_program():
    global _PROG
    if _PROG is None:
        import os
        _PROG = _build_program(int(os.environ.get("BASS_MHA_LOOP", "0")),
                               os.environ.get("BASS_MHA_VARIANT", ""))
    return _PROG


def make_in_maps(x, W_qkv, W_out):
    x = np.ascontiguousarray(np.asarray(x, dtype=np.float32))
    W_qkv = np.asarray(W_qkv, dtype=np.float32)
    W_out = np.asarray(W_out, dtype=np.float32)

    def bd(w):  # block_diag(w.T, w.T)
        out = np.zeros((128, 128), dtype=np.float16)
        out[0:64, 0:64] = w.T
        out[64:128, 64:128] = w.T
        return out

    bdq = bd(W_qkv[0:64])
    bdk = bd(W_qkv[64:128])
    bdv = bd(W_qkv[128:192])
    WoT = np.ascontiguousarray(W_out.T.astype(np.float16))
    in_maps = []
    for c in range(NCORES):
        in_maps.append({
            "xst": np.ascontiguousarray(
                x[:, :, c * 128:(c + 1) * 128].transpose(0, 2, 1)
            ).astype(np.float16),
            "bdq": bdq,
            "bdk": bdk,
            "bdv": bdv,
            "wot": np.ascontiguousarray(WoT[c * 128:(c + 1) * 128, :]),
        })
    return in_maps


def kernel(x, W_qkv, W_out, b_out, _trace=False):
    from concourse import bass_utils

    nc = _get_program()
    in_maps = make_in_maps(x, W_qkv, W_out)
    res = bass_utils.run_bass_kernel_spmd(
        nc, in_maps, core_ids=list(range(NCORES)), trace=_trace
    )
    acc = np.zeros((B, S, E), dtype=np.float64)
    for r in res.results:
        acc += r["part"]
    acc += np.asarray(b_out, dtype=np.float64)
    out = acc.astype(np.float32)
    if _trace:
        return out, res
    return out
